# revision 32
# baseline (speedup 1.0000x reference)
"""Trainium2 Bass kernel for nn_MultiHeadAttention (x:[2,2048,512], 8 heads, d=64).

Sharding: 8 cores = 2 batches x 4 head-pairs. Each core computes the QKV
projection for its 2 heads, the attention, and a partial (row-split) O
projection. Host sums the 4 bf16 partials per batch and adds the output bias.

Measured ~109-112us per core (baseline 141-174us). rel err 7.7e-3 (gate 2e-2).

Current design (v14):
 - everything 16-bit on the PE: x/wqkv/wo/Q/K/P/V/Z all bf16 (no float32r
   fp32_mode=HIGH multi-pass matmuls, no fp16 either - bf16 matmuls measure
   ~385ns vs 439ns fp16 for 512 moving cols and enable Fast Weight Load).
   Scores quadrant-packed: the two heads' K=64 matmuls run concurrently at
   tile_position (0,0)/(64,0).
 - depth-2 software pipeline: av(p) trails sc(p) by two steps so each exp
   (~1.0-1.2us per [128,1024] tile) has ~2 PE cadences of slack and never
   gates the PE. psum: 2 sc tiles (4 banks) + av accumulator (2) + freed
   prev-av reused by the O-projection (2) = 8 banks exactly. K/Q use
   separate psum tiles so the 2-slot ring parity stays even per qb block.
 - exp split alternates ScalarE ACT (exact, gamma-biased) and a custom 7-op
   DVE instruction (EXP_FAST_ANT): bf16-bit-trick exp with a factored
   quadratic octave correction (max rel err ~0.46%; the global gain gamma
   is absorbed by the ACT bias so softmax cancels it). The exp scale
   (128*log2e*SCALE) is folded into wq/wk host-side (sqrt each) so the
   score matmul emits DVE-ready pre-scaled logits.
 - V stationary is [ones(64) | V(64)]: psum partitions 0:64 accumulate the
   softmax denominators (64 identical copies), so normalize is just a
   reciprocal read straight from psum on 64 lanes (custom DVE ops only
   handle base-partition-0 psum reads on HW) + two tensor_muls; no
   partition broadcast, no single-partition ops.
 - loads: weights packed host-side into [128, 1536] bf16 (1KB lines per
   group); x streamed as per-qtile chunks (1KB lines) spread across the
   sync/scalar/gpsimd DMA queues, gate-ordered so the first K-proj matmul
   starts ~11us in (preamble ~7us + DMA ramp ~2.4us are fixed costs).
 - outputs staged to bf16 (halves DMA bytes + 2x faster psum evacuation);
   normalize + O-projection of the last q-tile run in 128-col chunks on 4
   distinct psum banks to shorten the drain, which lands in the HAM
   half-clock cooldown window (the core sustains ~78-89us of full-speed
   activity, then gates to 4/8 duty for 12-20us).
"""

import sys

import numpy as np

for _p in ("/opt/trn_rl_repo",):
    if _p not in sys.path:
        sys.path.insert(0, _p)

import ml_dtypes  # noqa: E402

import concourse.bass as bass  # noqa: E402
import concourse.tile as tile  # noqa: E402
from concourse import bacc, mybir  # noqa: E402
from concourse.bass_utils import run_bass_kernel_spmd  # noqa: E402

EMBED = 512
NH = 8
HD = 64
S = 2048
B = 2
SCALE = HD ** -0.5
F32 = mybir.dt.float32
F16 = mybir.dt.float16
BF16 = mybir.dt.bfloat16
I16 = mybir.dt.int16

N_KT = EMBED // 128   # 4 contraction k-tiles for the projections
N_QT = S // 512       # 4 q column tiles
N_ST = S // 128       # 16 seq tiles of 128

# ---- fast-exp constants (see module docstring; calibrated offline) ----
# pipeline: w = sp + FE_B; n = w & FE_MASK (floor-to-128 in bf16-bit units);
# g = w - n; out = n + (g*FE_C2)*(g + FE_CA); write-cast int16; bitcast bf16.
# Requires sp = raw_score * FE_A (folded into wq/wk host-side).
FE_A = float(np.float32(128 * np.log2(np.e) * SCALE))
FE_B = 17923.942462614166
FE_C2 = 0.0025459323352996915
FE_CA = 261.89949378833876
FE_LG = 9.031370392726972          # ln(gamma): global gain of the fast path
FE_MASK = 32640.0                  # bits 0x46FF0000: keeps exp=141 + top-7 mantissa
ACT_KAPPA = float(SCALE / FE_A)    # ACT path: exp(kappa*sp + FE_LG) = gamma*exp(SCALE*s)

# which (qt, ks) chunks run exp on the DVE custom op (rest on ScalarE).
# qt0 overlaps the QKV phase where the DVE is busy staging -> lighter share.
def _use_dve(qt: int, ks: int) -> bool:
    if qt == 0:
        return ks % 4 == 3
    return ks % 2 == 1


# ---- custom DVE op registration (idempotent) ----
def _register_exp_fast():
    import concourse.dve_ops as dvo
    from concourse.dve_spec import AluOp, Bin, C0, C1, C2, Spec, Src0, Src1, lower
    from concourse.dve_uop import DveOpSpec

    if "EXP_FAST_ANT" in dvo._SUB_OPCODE_FOR_NAME:
        return next(op for op in dvo.OPS if op.name == "EXP_FAST_ANT")

    # Src1 (= FE_B, a broadcast const tile) must be consumed at stage 1 —
    # late-stage Src1 reads hard-fault the DVE. C0 carries the late Ca.
    _w = Src0 + Src1
    _n = Bin(AluOp.BITWISE_AND, _w, C1)
    _g = _w - _n
    _body = _n + (_g * C2) * (_g + C0)

    def _ref(in0, in1, c0, c1, c2):
        f32 = np.float32
        w = (np.asarray(in0, f32) + np.asarray(in1, f32)).astype(f32)
        mask = np.asarray(np.asarray(c1, f32)).view(np.uint32)
        n = (w.view(np.uint32) & mask).view(f32)
        g = (w - n).astype(f32)
        z = ((g * f32(c2)).astype(f32) * (g + f32(c0)).astype(f32)).astype(f32)
        return (n + z).astype(f32)

    spec = Spec(body=_body, reference=_ref)
    row = dvo._CUSTOM_DVE_ROW_BASE + len(dvo.OPS)
    assert row < 0x20, "no free custom-DVE opcode row"
    sha = DveOpSpec(
        name="EXP_FAST_ANT", opcode=row, uops=lower(spec, ver="v3"), rd1_en=True
    ).sha("v3")
    op = dvo.DveOp("EXP_FAST_ANT", spec, subdim=False, uops_sha={"v3": sha})
    dvo.OPS.append(op)
    dvo.CUSTOM_DVE_SPECS[op.name] = op.spec
    dvo._SUB_OPCODE_FOR_NAME[op.name] = row
    return op


EXP_FAST_ANT = _register_exp_fast()


def build_nc():
    nc = bacc.Bacc("TRN2", target_bir_lowering=False, debug=False)

    xT_d = nc.dram_tensor("xT", [EMBED, S], BF16, kind="ExternalInput").ap()
    # packed per-partition: [wk(4x128) | wq(4x128) | wv(4x128)] = 3KB lines
    wqkv_d = nc.dram_tensor("wqkv", [128, 3 * N_KT * 128], BF16, kind="ExternalInput").ap()
    wo_d = nc.dram_tensor("wo", [128, EMBED], BF16, kind="ExternalInput").ap()
    out_d = nc.dram_tensor("out", [S, EMBED], BF16, kind="ExternalOutput").ap()

    with tile.TileContext(nc) as tc:
        with (
            tc.tile_pool(name="persist", bufs=1) as persist,
            tc.tile_pool(name="pt_pool", bufs=3) as pt_pool,
            tc.tile_pool(name="norm", bufs=2) as norm_pool,
            tc.tile_pool(name="ostage", bufs=4) as ostage,
            tc.tile_pool(name="ps", bufs=2, space="PSUM") as ps_pool,
            tc.tile_pool(name="av", bufs=2, space="PSUM") as av_pool,
        ):
            # ---- loads, gate-ordered across the three DMA queues so the
            # first QKV matmul is gated by ~384KB, not the whole 2.4MB ----
            wkv_sb = persist.tile([128, 3, N_KT, 128], BF16)
            wk_sb = wkv_sb[:, 0]
            wq_sb = wkv_sb[:, 1]
            wv_sb = wkv_sb[:, 2]
            wo_sb = persist.tile([128, EMBED], BF16)
            xT_sb = persist.tile([128, N_KT, S], BF16)  # [part, ktile, seq]
            xT_r = xT_d.rearrange("(t p) s -> p t s", p=128)

            wqkv_r = wqkv_d.rearrange("p (g t m) -> p g t m", g=3, t=N_KT)
            # gate-ordered loads: wk + x[kt01,q0] land first (K proj kt0/1),
            # then x[kt23,q0] / wq / wv, spread over the three DMA queues
            nc.sync.dma_start(out=wkv_sb[:, 0], in_=wqkv_r[:, 0])       # wk
            nc.scalar.dma_start(out=xT_sb[:, 0:2, 0:512], in_=xT_r[:, 0:2, 0:512])
            nc.gpsimd.dma_start(out=wkv_sb[:, 1], in_=wqkv_r[:, 1])     # wq
            nc.sync.dma_start(out=xT_sb[:, 2:4, 0:512], in_=xT_r[:, 2:4, 0:512])
            nc.gpsimd.dma_start(out=wkv_sb[:, 2], in_=wqkv_r[:, 2])     # wv
            nc.sync.dma_start(out=xT_sb[:, :, 512:1024], in_=xT_r[:, :, 512:1024])
            nc.scalar.dma_start(out=xT_sb[:, :, 1024:1536], in_=xT_r[:, :, 1024:1536])
            nc.gpsimd.dma_start(out=xT_sb[:, :, 1536:2048], in_=xT_r[:, :, 1536:2048])
            nc.scalar.dma_start(out=wo_sb, in_=wo_d)

            # constants; V padded to 128 stationary columns: NumWeights==128
            # + bf16 triggers the automatic Fast Weight Load. Columns 0:64
            # are all ones, so psum partitions 0:64 each accumulate a copy of
            # the softmax denominators — the reciprocal (a custom DVE op that
            # only handles base-partition-0 psum reads on HW) reads them
            # straight from psum; V values land at partitions 64:128.
            V_sb = persist.tile([128, N_ST, 2, 128], BF16)
            nc.vector.memset(V_sb[:, :, :, 0:64], 1.0)
            fb_sb = persist.tile([128, 1024], F32)
            nc.vector.memset(fb_sb, FE_B)
            lg_sb = persist.tile([128, 1], F32)
            nc.vector.memset(lg_sb, FE_LG)

            QT_sb = persist.tile([128, S], BF16)
            KT_sb = persist.tile([128, S], BF16)
            Z_sb = persist.tile([128, S], BF16)

            # ---- QKV projection emitter (phase A, folded into qt0's loop
            # so the in-order PE stream never stalls on a late x chunk) ----
            def emit_qkv(qb):
                qs = bass.ts(qb, 512)
                # K and Q in separate psum tiles: 4 qkv allocations + 4 sc
                # allocations per qb keep the 2-slot psum ring parity even,
                # so a qkv tile never lands on the slot of the sc emitted one
                # step earlier (whose exp is still ~1.2us out)
                for i, w_sb in enumerate((wk_sb, wq_sb)):
                    pkq = ps_pool.tile([128, 512], F32, tag="ps", name="pkq")
                    for kt in range(N_KT):
                        nc.tensor.matmul(
                            pkq, w_sb[:, kt, :], xT_sb[:, kt, qs],
                            start=(kt == 0), stop=(kt == N_KT - 1),
                        )
                    dst = KT_sb if i == 0 else QT_sb
                    nc.vector.tensor_copy(dst[:, qs], pkq)
                # V projection: two seq tiles of 128 per psum tile
                for sp_i in range(2):
                    psv = ps_pool.tile([128, 2, 512], F32, tag="ps")
                    for j in range(2):
                        st = 4 * qb + 2 * sp_i + j
                        for kt in range(N_KT):
                            nc.tensor.matmul(
                                psv[:, j, 0:128],
                                xT_sb[:, kt, bass.ts(st, 128)],
                                wv_sb[:, kt, :],
                                start=(kt == 0), stop=(kt == N_KT - 1),
                            )
                    for j in range(2):
                        st = 4 * qb + 2 * sp_i + j
                        # two contiguous [128,64] copies beat one strided
                        # [128,2,64] copy on the DVE
                        for h in range(2):
                            nc.vector.tensor_copy(
                                V_sb[:, st, h, HD:128],
                                psv[:, j, h * HD:(h + 1) * HD],
                            )

            # ---- attention, software-pipelined one iteration ahead:
            # PE order [sc(p), av(p-1), sc(p+1), av(p), ...] so the ACT and
            # DVE exp instructions for consecutive iterations overlap ----
            av_tiles = []
            sc_tiles = {}
            pt_tiles = {}

            def emit_oproj_one(src_qt, mi):
                m = 4 * src_qt + mi
                po = av_tiles[src_qt][:, mi % 2, :]  # reuse freed av psum bank
                nc.tensor.matmul(
                    po, Z_sb[:, bass.ts(m, 128)], wo_sb, start=True, stop=True,
                )
                ot = ostage.tile([128, 512], BF16, tag="ot")
                if mi % 2 == 0:
                    nc.scalar.copy(ot, po)
                else:
                    nc.vector.tensor_copy(ot, po)
                nc.sync.dma_start(out=out_d[bass.ts(m, 128), :], in_=ot)

            NP = N_QT * N_ST   # 64 iterations

            def emit_sc_exp(p):
                qt, ks = divmod(p, N_ST)
                qs = bass.ts(qt, 512)
                kk = bass.ts(ks, 128)
                sc = ps_pool.tile([128, 2, 512], F32, tag="ps")
                sc_tiles[p] = sc
                nc.tensor.matmul(
                    sc[:, 0, :], KT_sb[0:64, kk], QT_sb[0:64, qs],
                    start=True, stop=True, tile_position=(0, 0),
                )
                nc.tensor.matmul(
                    sc[:, 1, :], KT_sb[64:128, kk], QT_sb[64:128, qs],
                    start=True, stop=True, tile_position=(64, 0),
                )
                pt = pt_pool.tile([128, 2, 512], BF16, tag="pt")
                pt_tiles[p] = pt
                if _use_dve(qt, ks):
                    nc.vector._custom_dve(
                        EXP_FAST_ANT,
                        out=pt.bitcast(I16),
                        in0=sc,
                        in1=fb_sb,
                        s0=FE_CA,
                        s1=FE_MASK,
                        imm2=FE_C2,
                    )
                else:
                    nc.scalar.activation(
                        out=pt, in_=sc,
                        func=mybir.ActivationFunctionType.Exp,
                        scale=ACT_KAPPA, bias=lg_sb,
                    )

            def emit_av(p):
                qt, ks = divmod(p, N_ST)
                if ks == 0:
                    av_tiles.append(
                        av_pool.tile([128, 2, 512], F32, tag="av", name="av")
                    )
                av = av_tiles[qt]
                pt = pt_tiles.pop(p)
                # M=128 (padded): rows 0:64 accumulate V^T @ PT, row 64 (ones
                # col) the softmax denominators, rows 65:128 harmless filler
                for h in range(2):
                    nc.tensor.matmul(
                        av[:, h, :], V_sb[:, ks, h, :], pt[:, h, :],
                        start=(ks == 0), stop=(ks == N_ST - 1),
                    )
            # normalize split into pieces staggered across the NEXT q-tile's
            # first iterations, so the DVE FIFO never delays an exp by more
            # than ~1 piece. The stationary ones-columns 64:128 mean psum
            # partitions 64:128 each hold a copy of the softmax denominators,
            # so the reciprocal runs on 64 lanes straight from psum — no
            # single-partition copy and no gpsimd partition_broadcast.
            def make_normalize_pieces(qt):
                qs = bass.ts(qt, 512)
                av = av_tiles[qt]
                ctx = {}

                def p0():
                    rb_sb = norm_pool.tile([64, 2, 512], F32, tag="rb", name="rb")
                    nc.vector.reciprocal_approx_fast(out=rb_sb, in_=av[0:64, :, :])
                    ctx["rb"] = rb_sb

                def p1():
                    nc.vector.tensor_mul(
                        Z_sb[0:64, qs], av[64:128, 0, :], ctx["rb"][0:64, 0, :]
                    )

                def p2():
                    nc.vector.tensor_mul(
                        Z_sb[64:128, qs], av[64:128, 1, :], ctx["rb"][0:64, 1, :]
                    )

                return [p0, p1, p2]

            # depth-2 software pipeline: av(p) trails sc(p) by two steps so
            # each exp has ~2 PE cadences of slack and never gates the PE
            # (at depth 1 the 1220ns DVE exp exceeded the 1141ns PE budget).
            # psum: 2 sc tiles in flight (4 banks) + av accumulator (2) +
            # freed prev-av reused by oproj (2) = 8 banks exactly.
            pending = []
            for step in range(NP + 2):
                qt, ks = divmod(step, N_ST)
                # normalize pieces run 2-at-a-time on even-ks steps, where
                # the exp lands on ACT and the DVE is free
                if pending and ks % 2 == 0:
                    pending.pop(0)()
                    if pending:
                        pending.pop(0)()
                if step < NP and qt == 0 and ks % 4 == 0:
                    emit_qkv(ks // 4)
                if step < NP and qt >= 1 and ks in (4, 6, 8, 10):
                    emit_oproj_one(qt - 1, (ks - 4) // 2)
                if step < NP:
                    emit_sc_exp(step)
                j = step - 2
                if j >= 0:
                    emit_av(j)
                    closed = (j + 1) // N_ST - 1
                    if (j + 1) % N_ST == 0 and 0 <= closed < N_QT - 1:
                        pending.extend(make_normalize_pieces(closed))
            # last q-tile: normalize and O-project in 128-column chunks so
            # the first tail matmul starts after 1/4 of the muls, on 4
            # distinct psum banks (freed av + sc buffers); copies split
            # ACT/DVE and the DMAs split across both rings
            qt3 = N_QT - 1
            av3 = av_tiles[qt3]
            rb_sb = norm_pool.tile([64, 2, 512], F32, tag="rb", name="rb")
            nc.vector.reciprocal_approx_fast(out=rb_sb, in_=av3[0:64, :, :])
            po_t = av_pool.tile([128, 2, 512], F32, tag="av", name="avt")
            ps_t = ps_pool.tile([128, 2, 512], F32, tag="ps", name="pst")
            slots = [po_t[:, 0, :], po_t[:, 1, :], ps_t[:, 0, :], ps_t[:, 1, :]]
            for c in range(4):
                m = 4 * qt3 + c
                mc = bass.ts(m, 128)
                lc = bass.ts(c, 128)
                nc.vector.tensor_mul(
                    Z_sb[0:64, mc], av3[64:128, 0, lc], rb_sb[0:64, 0, lc]
                )
                nc.vector.tensor_mul(
                    Z_sb[64:128, mc], av3[64:128, 1, lc], rb_sb[0:64, 1, lc]
                )
                nc.tensor.matmul(
                    slots[c], Z_sb[:, mc], wo_sb, start=True, stop=True,
                )
            ots = []
            for mi in range(4):
                ot = ostage.tile([128, 512], BF16, tag="ot", name="ot")
                ots.append(ot)
                if mi % 2 == 0:
                    nc.scalar.copy(ot, slots[mi])
                else:
                    nc.vector.tensor_copy(ot, slots[mi])
            for mi in range(4):
                m = 4 * qt3 + mi
                ring = nc.scalar if mi % 2 == 0 else nc.sync
                ring.dma_start(out=out_d[bass.ts(m, 128), :], in_=ots[mi])

    nc.compile()
    return nc


_NC = None


def _get_nc():
    global _NC
    if _NC is None:
        _NC = build_nc()
    return _NC


def _pack_w(w):
    """[512, 128] -> [128 part, 4 ktile, 128 col] bf16."""
    return w.reshape(N_KT, 128, 128).transpose(1, 0, 2).astype(ml_dtypes.bfloat16)


def make_in_maps(x, w_qkv, w_o):
    x = np.ascontiguousarray(np.asarray(x, dtype=np.float32))
    w_qkv = np.asarray(w_qkv, dtype=np.float32)
    w_o = np.asarray(w_o, dtype=np.float32)
    in_maps = []
    xTs = [np.ascontiguousarray(x[b].T.astype(ml_dtypes.bfloat16)) for b in range(B)]
    # sqrt(FE_A) folded into BOTH wq and wk keeps fp16 weights in range
    sa = np.float32(np.sqrt(FE_A))
    for c in range(8):
        b, g = c // 4, c % 4
        cols = slice(2 * g * HD, (2 * g + 2) * HD)
        wk_p = _pack_w(w_qkv[:, EMBED:2 * EMBED][:, cols] * sa)
        wq_p = _pack_w(w_qkv[:, :EMBED][:, cols] * sa)
        wv_p = _pack_w(w_qkv[:, 2 * EMBED:][:, cols])
        wqkv = np.stack([wk_p, wq_p, wv_p], axis=1)  # [128, 3, 4, 128]
        in_maps.append({
            "xT": xTs[b],
            "wqkv": np.ascontiguousarray(wqkv.reshape(128, 3 * N_KT * 128)),
            "wo": np.ascontiguousarray(
                w_o[cols, :].astype(ml_dtypes.bfloat16)
            ),
        })
    return in_maps


def combine(results, b_o):
    partials = np.stack([np.asarray(r["out"], dtype=np.float32) for r in results])
    out = partials.reshape(B, 4, S, EMBED).sum(axis=1)
    return (out + np.asarray(b_o, dtype=np.float32)).astype(np.float32)


def kernel(x, w_qkv, w_o, b_o):
    nc = _get_nc()
    res = run_bass_kernel_spmd(nc, make_in_maps(x, w_qkv, w_o), core_ids=list(range(8)))
    return combine(res.results, b_o)
